# revision 25
# baseline (speedup 1.0000x reference)
"""Bayesian SSM kernel for 8 Trainium2 NeuronCores.

Math (reference, with d_state N == d_model D):
    A  = -exp(x @ WA.T + bA)        dt = exp(x @ Wdt.T + bdt)
    E  = exp(A*dt) = exp(-exp(x @ (WA+Wdt).T + (bA+bdt)))   <- fuses 2 projections into 1
    Bx = (x @ WB.T + bB) * x
    h_t = E_t * h_{t-1} + Bx_t      y_t = (x @ WC.T + bC)_t * h_t * batch_size

The Bayesian weight sampling (w = mu + exp(0.5*lv)*eps, fixed key 42) is
input-independent, so it is reproduced bit-exactly on host with jax CPU.

Sharding over 8 cores: (batch b in 0..3) x (channel half j in 0..1). Each core
computes y[b, :, j*512:(j+1)*512]. On-device layout is channel-on-partition,
time-on-free: projections via PE matmuls (bf16, fp32 accum), decay/injection
via ACT/DVE, the recurrence via the native tensor_tensor_scan instruction
along the free (time) axis.

DMA is lean: per core two bf16 DRAM tensors — x^T [1024, 4096] and the packed
transposed weight blocks WS^T | (4*WB)^T | WC^T [1024, 1536] — with their 1024
contraction rows permuted so the core's own channel-half chunks come first.
That lets the injection (Bm*x) reuse the bf16 x tiles already resident for the
matmuls (chunk n == own-half channels [n*128,(n+1)*128)). The *batch_size(=4)
output factor is folded into WB/bB (exact: power-of-two scaling). DMA issue
order (bias, then per-chunk weights + x quarter 0, then remaining x quarters)
minimizes the cold-start PE stall; y tiles stream out on the second HWDGE
ring as they are produced.
"""

import numpy as np
import ml_dtypes

B, L, D, N = 4, 4096, 1024, 1024
HALF = N // 2          # channels per core
P = 128                # partitions
KC = D // P            # contraction chunks (8)
NN = HALF // P         # output-channel chunks per core (4)
TT = 512               # time-tile (free dim per matmul / PSUM bank)
NT = L // TT           # time tiles (8)
XW = L + 3 * HALF      # packed bf16 tensor width (5632)

_STATE = {}

XQ = 1024              # x DMA column-split width (per-quarter)
NQ = L // XQ           # 4 column quarters


def _build_nc(loop_k: int = 1, pe_only: bool = False, dma_only: bool = False,
              one_group: bool = False):
    import contextlib

    import concourse.bacc as bacc
    import concourse.mybir as mybir
    import concourse.tile as tile
    from concourse.bass import ts

    f32 = mybir.dt.float32
    bf16 = mybir.dt.bfloat16
    Exp = mybir.ActivationFunctionType.Exp
    add = mybir.AluOpType.add
    mult = mybir.AluOpType.mult

    nc = bacc.Bacc("TRN2", target_bir_lowering=False)

    xin = nc.dram_tensor("xin", [D, L], bf16, kind="ExternalInput")
    win = nc.dram_tensor("win", [D, 3 * HALF], bf16, kind="ExternalInput")
    bias = nc.dram_tensor("bias", [P, 3 * NN], f32, kind="ExternalInput")
    y = nc.dram_tensor("y", [HALF, L], f32, kind="ExternalOutput")

    x_t = xin.rearrange("(k p) t -> k p t", p=P)
    w_t = win.rearrange("(k p) n -> k p n", p=P)
    y_t = y.rearrange("(n p) t -> n p t", p=P)

    with tile.TileContext(nc) as tc:
        # loop_k > 1 repeats the entire body inside the NEFF (bench-only:
        # amortizes per-call dispatch overhead to measure the body's HW time)
        loop_cm = tc.For_i(0, loop_k, 1) if loop_k > 1 else contextlib.nullcontext()
        with (
            tc.tile_pool(name="res", bufs=1) as res,
            tc.tile_pool(name="work", bufs=3) as work,
            tc.tile_pool(name="hpool", bufs=2) as hpool,
            tc.tile_pool(name="ypool", bufs=3) as ypool,
            tc.tile_pool(name="psum", bufs=2, space="PSUM") as psum,
        ):
            def load_inputs():
                # DMA issue order puts what the first matmul groups need
                # (bias, S-layer weights + x quarter 0, chunk-k-major) first,
                # so the PE starts a few us into a cold NEFF instead of ~26us.
                nonlocal bias_sb, w_sb, x_sb
                bias_sb = res.tile([P, 3 * NN], f32, tag="bias")
                nc.sync.dma_start(bias_sb[:], bias[:])
                w_sb, x_sb = {}, {}
                for k in range(KC):
                    w_ = res.tile([P, HALF], bf16, tag=f"wS{k}")
                    nc.sync.dma_start(w_[:], w_t[k, :, ts(0, HALF)])
                    w_sb[0, k] = w_
                    t_ = res.tile([P, XQ], bf16, tag=f"x{k}_0")
                    nc.sync.dma_start(t_[:], x_t[k, :, ts(0, XQ)])
                    x_sb[k, 0] = t_
                for k in range(KC):
                    w_ = res.tile([P, 2 * HALF], bf16, tag=f"wBC{k}")
                    nc.sync.dma_start(w_[:], w_t[k, :, HALF:3 * HALF])
                    w_sb[1, k] = w_
                for q in range(1, NQ):
                    for k in range(KC):
                        t_ = res.tile([P, XQ], bf16, tag=f"x{k}_{q}")
                        nc.sync.dma_start(t_[:], x_t[k, :, ts(q, XQ)])
                        x_sb[k, q] = t_

            bias_sb = w_sb = x_sb = None
            if pe_only:
                load_inputs()  # once, outside the loop

            def wsl(layer, k, n):
                # lhsT slice for layer l (0=S,1=B4,2=C): [128 K x 128 M]
                if layer == 0:
                    return w_sb[0, k][:, ts(n, P)]
                return w_sb[1, k][:, (layer - 1) * HALF + n * P:(layer - 1) * HALF + (n + 1) * P]

            def xsl(k, t):
                # rhs slice [128 K x 512 t] of time-tile t
                return x_sb[k, t // 2][:, (t % 2) * TT:(t % 2 + 1) * TT]

            with loop_cm:
                if not pe_only:
                    load_inputs()
                if dma_only:
                    for n in range(NN):
                        for t in range(NT):
                            yd = ypool.tile([P, TT], f32, tag="yt")
                            nc.vector.tensor_copy(yd[:], x_sb[n, t // 2][:, (t % 2) * TT:(t % 2 + 1) * TT])
                            nc.scalar.dma_start(y_t[n, :, ts(t, TT)], yd[:])
                    continue_main = []
                else:
                    continue_main = list(range(NN))

                if pe_only and one_group:
                    # probe: one giant accumulation group, no psum handoffs
                    ps = psum.tile([P, TT], f32, tag="S")
                    nmm = NN * NT * 3 * KC
                    i = 0
                    for n in range(NN):
                        for t in range(NT):
                            for layer in range(3):
                                for k in range(KC):
                                    nc.tensor.matmul(ps[:], wsl(layer, k, n), xsl(k, t),
                                                     start=(i == 0), stop=(i == nmm - 1))
                                    i += 1
                    continue_main = []

                for n in continue_main:
                    hprev = None
                    for t in range(NT):
                        psS = psum.tile([P, TT], f32, tag="S")
                        psB = psum.tile([P, TT], f32, tag="B")
                        psC = psum.tile([P, TT], f32, tag="C")
                        for k in range(KC):
                            nc.tensor.matmul(psS[:], wsl(0, k, n), xsl(k, t),
                                             start=(k == 0), stop=(k == KC - 1))
                        for k in range(KC):
                            nc.tensor.matmul(psB[:], wsl(1, k, n), xsl(k, t),
                                             start=(k == 0), stop=(k == KC - 1))
                        for k in range(KC):
                            nc.tensor.matmul(psC[:], wsl(2, k, n), xsl(k, t),
                                             start=(k == 0), stop=(k == KC - 1))

                        if pe_only:
                            continue

                        # E = exp(-exp(S + bS))
                        e1 = work.tile([P, TT], f32, tag="e1")
                        nc.scalar.activation(e1[:], psS[:], Exp, bias=bias_sb[:, n:n + 1])
                        Et = work.tile([P, TT], f32, tag="E")
                        nc.scalar.activation(Et[:], e1[:], Exp, scale=-1.0)

                        # 4*Bx = (4B + 4bB) * x   (the 4 is folded into WB/bB)
                        bx = work.tile([P, TT], f32, tag="bx")
                        nc.vector.scalar_tensor_tensor(bx[:], psB[:], bias_sb[:, NN + n:NN + n + 1],
                                                       xsl(n, t), op0=add, op1=mult)

                        # h_t = E_t*h_{t-1} + Bx_t along the free/time axis
                        ht = hpool.tile([P, TT], f32, tag="h")
                        init = 0.0 if t == 0 else hprev[:, TT - 1:TT]
                        nc.vector.tensor_tensor_scan(ht[:], Et[:], bx[:], init,
                                                     op0=mult, op1=add)
                        hprev = ht

                        # y = (C + bC) * h; store each tile as it's produced
                        # (scalar-ring DMA, hidden under PE; keeps the kernel
                        # tail short)
                        yt = ypool.tile([P, TT], f32, tag="yt")
                        nc.vector.scalar_tensor_tensor(yt[:], psC[:],
                                                       bias_sb[:, 2 * NN + n:2 * NN + n + 1],
                                                       ht[:], op0=add, op1=mult)
                        nc.scalar.dma_start(y_t[n, :, ts(t, TT)], yt[:])

    nc.compile()
    return nc


def _sample_weights(inputs):
    """Reproduce the reference's Bayesian weight sampling bit-exactly."""
    import jax
    import jax.numpy as jnp

    cpu = jax.devices("cpu")[0]
    with jax.default_device(cpu):
        key = jax.random.key(42)
        kA, kB, kC, kdt = jax.random.split(key, 4)

        def sample(prefix, k):
            kw, kb = jax.random.split(k)
            wmu, wlv = inputs[f"{prefix}_wmu"], inputs[f"{prefix}_wlv"]
            bmu, blv = inputs[f"{prefix}_bmu"], inputs[f"{prefix}_blv"]
            w = wmu + np.exp(0.5 * wlv) * np.asarray(
                jax.random.normal(kw, wmu.shape, jnp.float32))
            b = bmu + np.exp(0.5 * blv) * np.asarray(
                jax.random.normal(kb, bmu.shape, jnp.float32))
            return w.astype(np.float32), b.astype(np.float32)

        WA, bA = sample("A", kA)
        WB, bB = sample("Bp", kB)
        WC, bC = sample("C", kC)
        Wdt, bdt = sample("dt", kdt)
    return (WA + Wdt, bA + bdt), (4.0 * WB, 4.0 * bB), (WC, bC)


def _prep_in_maps(inputs):
    (WS, bS), (WB4, bB4), (WC, bC) = _sample_weights(inputs)
    x = np.ascontiguousarray(inputs["x"], dtype=np.float32)

    xT_bf = [x[b].T.astype(ml_dtypes.bfloat16) for b in range(B)]
    # weight blocks in [d, n] (lhsT) layout, bf16
    wT = {}
    for j in range(2):
        sl = slice(j * HALF, (j + 1) * HALF)
        wT[j] = np.concatenate(
            [W[sl].T.astype(ml_dtypes.bfloat16) for W in (WS, WB4, WC)], axis=1)

    def bcol(bias_vec, j):
        return bias_vec[j * HALF:(j + 1) * HALF].reshape(NN, P).T  # [128, NN]

    in_maps = []
    for c in range(8):
        b, j = c // 2, c % 2
        # contraction-chunk permutation: own channel-half chunks first
        perm = list(range(4 * j, 4 * j + 4)) + list(range(4 * (1 - j), 4 * (1 - j) + 4))
        row_perm = np.concatenate([np.arange(k * P, (k + 1) * P) for k in perm])
        bias_m = np.concatenate(
            [bcol(v, j) for v in (bS, bB4, bC)], axis=1).astype(np.float32)
        in_maps.append({"xin": np.ascontiguousarray(xT_bf[b][row_perm]),
                        "win": np.ascontiguousarray(wT[j][row_perm]),
                        "bias": np.ascontiguousarray(bias_m)})
    return in_maps


def kernel(**inputs) -> np.ndarray:
    from concourse.bass_utils import run_bass_kernel_spmd

    if "nc" not in _STATE:
        _STATE["nc"] = _build_nc()
    nc = _STATE["nc"]

    in_maps = _prep_in_maps(inputs)
    res = run_bass_kernel_spmd(nc, in_maps, core_ids=list(range(8)),
                               trace=bool(_STATE.get("trace")))
    _STATE["last_results"] = res

    out = np.empty((B, L, N), np.float32)
    for c in range(8):
        b, j = c // 2, c % 2
        out[b, :, j * HALF:(j + 1) * HALF] = res.results[c]["y"].T
    return out


# revision 32
# speedup vs baseline: 1.0164x; 1.0164x over previous
"""Bayesian SSM kernel for 8 Trainium2 NeuronCores.

Math (reference, with d_state N == d_model D):
    A  = -exp(x @ WA.T + bA)        dt = exp(x @ Wdt.T + bdt)
    E  = exp(A*dt) = exp(-exp(x @ (WA+Wdt).T + (bA+bdt)))   <- fuses 2 projections into 1
    Bx = (x @ WB.T + bB) * x
    h_t = E_t * h_{t-1} + Bx_t      y_t = (x @ WC.T + bC)_t * h_t * batch_size

The Bayesian weight sampling (w = mu + exp(0.5*lv)*eps, fixed key 42) is
input-independent, so it is reproduced bit-exactly on host with jax CPU.

Sharding over 8 cores: (batch b in 0..3) x (channel half j in 0..1). Each core
computes y[b, :, j*512:(j+1)*512]. On-device layout is channel-on-partition,
time-on-free: projections via PE matmuls (bf16, fp32 accum), decay/injection
via ACT/DVE, the recurrence via the native tensor_tensor_scan instruction
along the free (time) axis.

DMA is lean: per core two bf16 DRAM tensors — x^T [1024, 4096] and the packed
transposed weight blocks WS^T | (4*WB)^T | WC^T [1024, 1536] — with their 1024
contraction rows permuted so the core's own channel-half chunks come first.
That lets the injection (Bm*x) reuse the bf16 x tiles already resident for the
matmuls (chunk n == own-half channels [n*128,(n+1)*128)). The *batch_size(=4)
output factor is folded into WB/bB (exact: power-of-two scaling). DMA issue
order (bias, then per-chunk weights + x quarter 0, then remaining x quarters)
minimizes the cold-start PE stall; y tiles stream out on the second HWDGE
ring as they are produced.
"""

import numpy as np
import ml_dtypes

B, L, D, N = 4, 4096, 1024, 1024
HALF = N // 2          # channels per core
P = 128                # partitions
KC = D // P            # contraction chunks (8)
NN = HALF // P         # output-channel chunks per core (4)
TT = 512               # time-tile (free dim per matmul / PSUM bank)
NT = L // TT           # time tiles (8)

_STATE = {}

XQ = 1024              # x DMA column-split width (per-quarter)
NQ = L // XQ           # 4 column quarters


def _dedup_ldweights(bir_json: bytes) -> bytes:
    """Drop redundant LDWEIGHTS from the BIR: Tile's legalization emits one
    explicit Ldweights per Matmult (walrus lowers them 1:1); when consecutive
    matmuls use the bit-identical stationary operand, the reload is a no-op
    on the PE array. Only wait/update-free repeats separated by nothing but
    Matmults are removed, so all semaphore semantics are preserved."""
    import json

    m = json.loads(bir_json)
    removed = 0
    for f in m["functions"]:
        for blk in f["blocks"]:
            out, last_sig = [], None
            for inst in blk["instructions"]:
                op = inst.get("opcode")
                if op == "Ldweights":
                    si = inst.get("sync_info") or {}
                    sig = json.dumps(inst.get("ins"), sort_keys=True)
                    if (sig == last_sig and not si.get("on_wait")
                            and not si.get("on_update")):
                        removed += 1
                        continue
                    last_sig = sig
                elif op != "Matmult":
                    last_sig = None
                out.append(inst)
            blk["instructions"] = out
    if not removed:
        return bir_json
    return json.dumps(m).encode()


def _install_ldw_dedup():
    """Route every BIR->NEFF compile through _dedup_ldweights."""
    if _STATE.get("ldw_patch"):
        return
    import concourse.bass_utils as bu
    import concourse.bass2jax as b2j

    orig = bu.compile_bir_kernel

    def patched(bir_json, tmpdir, neff_name="file.neff"):
        return orig(_dedup_ldweights(bir_json), tmpdir, neff_name)

    bu.compile_bir_kernel = patched
    b2j.compile_bir_kernel = patched
    _STATE["ldw_patch"] = True


def _build_nc(loop_k: int = 1, pe_only: bool = False, dma_only: bool = False,
              one_group: bool = False):
    import contextlib

    import concourse.bacc as bacc
    import concourse.mybir as mybir
    import concourse.tile as tile
    from concourse.bass import ts

    f32 = mybir.dt.float32
    bf16 = mybir.dt.bfloat16
    Exp = mybir.ActivationFunctionType.Exp
    add = mybir.AluOpType.add
    mult = mybir.AluOpType.mult

    nc = bacc.Bacc("TRN2", target_bir_lowering=False)

    xin = nc.dram_tensor("xin", [D, L], bf16, kind="ExternalInput")
    win = nc.dram_tensor("win", [D, 3 * HALF], bf16, kind="ExternalInput")
    bias = nc.dram_tensor("bias", [P, 3 * NN], f32, kind="ExternalInput")
    y = nc.dram_tensor("y", [HALF, L], f32, kind="ExternalOutput")

    x_t = xin.rearrange("(k p) t -> k p t", p=P)
    w_t = win.rearrange("(k p) n -> k p n", p=P)
    y_t = y.rearrange("(n p) t -> n p t", p=P)

    with tile.TileContext(nc) as tc:
        # loop_k > 1 repeats the entire body inside the NEFF (bench-only:
        # amortizes per-call dispatch overhead to measure the body's HW time)
        loop_cm = tc.For_i(0, loop_k, 1) if loop_k > 1 else contextlib.nullcontext()
        with (
            tc.tile_pool(name="res", bufs=1) as res,
            tc.tile_pool(name="work", bufs=3) as work,
            tc.tile_pool(name="hpool", bufs=2) as hpool,
            tc.tile_pool(name="ypool", bufs=3) as ypool,
            tc.tile_pool(name="psum", bufs=1, space="PSUM") as psum,
        ):
            def load_inputs():
                # DMA issue order puts what the first matmul groups need
                # (bias, S-layer weights + x quarter 0, chunk-k-major) first,
                # so the PE starts a few us into a cold NEFF instead of ~26us.
                nonlocal bias_sb, w_sb, x_sb
                bias_sb = res.tile([P, 3 * NN], f32, tag="bias")
                nc.sync.dma_start(bias_sb[:], bias[:])
                w_sb, x_sb = {}, {}
                for k in range(KC):
                    w_ = res.tile([P, HALF], bf16, tag=f"wS{k}")
                    nc.sync.dma_start(w_[:], w_t[k, :, ts(0, HALF)])
                    w_sb[0, k] = w_
                    t_ = res.tile([P, XQ], bf16, tag=f"x{k}_0")
                    nc.sync.dma_start(t_[:], x_t[k, :, ts(0, XQ)])
                    x_sb[k, 0] = t_
                for k in range(KC):
                    w_ = res.tile([P, 2 * HALF], bf16, tag=f"wBC{k}")
                    nc.sync.dma_start(w_[:], w_t[k, :, HALF:3 * HALF])
                    w_sb[1, k] = w_
                for q in range(1, NQ):
                    for k in range(KC):
                        t_ = res.tile([P, XQ], bf16, tag=f"x{k}_{q}")
                        nc.sync.dma_start(t_[:], x_t[k, :, ts(q, XQ)])
                        x_sb[k, q] = t_

            bias_sb = w_sb = x_sb = None
            if pe_only:
                load_inputs()  # once, outside the loop

            def wsl(layer, k, n):
                # lhsT slice for layer l (0=S,1=B4,2=C): [128 K x 128 M]
                if layer == 0:
                    return w_sb[0, k][:, ts(n, P)]
                return w_sb[1, k][:, (layer - 1) * HALF + n * P:(layer - 1) * HALF + (n + 1) * P]

            def xsl(k, t):
                # rhs slice [128 K x 512 t] of time-tile t
                return x_sb[k, t // 2][:, (t % 2) * TT:(t % 2 + 1) * TT]

            with loop_cm:
                if not pe_only:
                    load_inputs()
                if dma_only:
                    for n in range(NN):
                        for t in range(NT):
                            yd = ypool.tile([P, TT], f32, tag="yt")
                            nc.vector.tensor_copy(yd[:], x_sb[n, t // 2][:, (t % 2) * TT:(t % 2 + 1) * TT])
                            nc.scalar.dma_start(y_t[n, :, ts(t, TT)], yd[:])
                    continue_main = []
                else:
                    continue_main = list(range(NN))

                if pe_only and one_group:
                    # probe: one giant accumulation group, no psum handoffs
                    ps = psum.tile([P, TT], f32, tag="S")
                    nmm = NN * NT * 3 * KC
                    i = 0
                    for n in range(NN):
                        for t in range(NT):
                            for layer in range(3):
                                for k in range(KC):
                                    nc.tensor.matmul(ps[:], wsl(layer, k, n), xsl(k, t),
                                                     start=(i == 0), stop=(i == nmm - 1))
                                    i += 1
                    continue_main = []

                for n in continue_main:
                    hprev = None
                    for tp in range(NT // 2):
                        t0 = 2 * tp
                        # t-PAIR structure: each weight slice feeds two
                        # consecutive matmuls (time tiles t0, t0+1), so the
                        # BIR ldweights-dedup pass (see _install_ldw_dedup)
                        # can drop every second LDWEIGHTS -> ~35us less PE time
                        ps = {}
                        for lname in "SBC":
                            for i in (0, 1):
                                ps[lname, i] = psum.tile([P, TT], f32, tag=f"{lname}{i}",
                                                         name=f"ps_{lname}{i}")
                        for layer, lname in ((0, "S"), (1, "B"), (2, "C")):
                            for k in range(KC):
                                nc.tensor.matmul(ps[lname, 0][:], wsl(layer, k, n), xsl(k, t0),
                                                 start=(k == 0), stop=(k == KC - 1))
                                nc.tensor.matmul(ps[lname, 1][:], wsl(layer, k, n), xsl(k, t0 + 1),
                                                 start=(k == 0), stop=(k == KC - 1))

                        if pe_only:
                            continue

                        for i in (0, 1):
                            t = t0 + i
                            psS, psB, psC = ps["S", i], ps["B", i], ps["C", i]
                            # E = exp(-exp(S + bS))
                            e1 = work.tile([P, TT], f32, tag="e1")
                            nc.scalar.activation(e1[:], psS[:], Exp, bias=bias_sb[:, n:n + 1])
                            Et = work.tile([P, TT], f32, tag="E")
                            nc.scalar.activation(Et[:], e1[:], Exp, scale=-1.0)

                            # 4*Bx = (4B + 4bB) * x   (the 4 is folded into WB/bB)
                            bx = work.tile([P, TT], f32, tag="bx")
                            nc.vector.scalar_tensor_tensor(bx[:], psB[:], bias_sb[:, NN + n:NN + n + 1],
                                                           xsl(n, t), op0=add, op1=mult)

                            # h_t = E_t*h_{t-1} + Bx_t along the free/time axis
                            ht = hpool.tile([P, TT], f32, tag="h")
                            init = 0.0 if t == 0 else hprev[:, TT - 1:TT]
                            nc.vector.tensor_tensor_scan(ht[:], Et[:], bx[:], init,
                                                         op0=mult, op1=add)
                            hprev = ht

                            # y = (C + bC) * h; store each tile as it's
                            # produced (scalar-ring DMA, hidden under PE)
                            yt = ypool.tile([P, TT], f32, tag="yt")
                            nc.vector.scalar_tensor_tensor(yt[:], psC[:],
                                                           bias_sb[:, 2 * NN + n:2 * NN + n + 1],
                                                           ht[:], op0=add, op1=mult)
                            nc.scalar.dma_start(y_t[n, :, ts(t, TT)], yt[:])

    nc.compile()
    return nc


def _sample_weights(inputs):
    """Reproduce the reference's Bayesian weight sampling bit-exactly."""
    import jax
    import jax.numpy as jnp

    cpu = jax.devices("cpu")[0]
    with jax.default_device(cpu):
        key = jax.random.key(42)
        kA, kB, kC, kdt = jax.random.split(key, 4)

        def sample(prefix, k):
            kw, kb = jax.random.split(k)
            wmu, wlv = inputs[f"{prefix}_wmu"], inputs[f"{prefix}_wlv"]
            bmu, blv = inputs[f"{prefix}_bmu"], inputs[f"{prefix}_blv"]
            w = wmu + np.exp(0.5 * wlv) * np.asarray(
                jax.random.normal(kw, wmu.shape, jnp.float32))
            b = bmu + np.exp(0.5 * blv) * np.asarray(
                jax.random.normal(kb, bmu.shape, jnp.float32))
            return w.astype(np.float32), b.astype(np.float32)

        WA, bA = sample("A", kA)
        WB, bB = sample("Bp", kB)
        WC, bC = sample("C", kC)
        Wdt, bdt = sample("dt", kdt)
    return (WA + Wdt, bA + bdt), (4.0 * WB, 4.0 * bB), (WC, bC)


def _prep_in_maps(inputs):
    (WS, bS), (WB4, bB4), (WC, bC) = _sample_weights(inputs)
    x = np.ascontiguousarray(inputs["x"], dtype=np.float32)

    xT_bf = [x[b].T.astype(ml_dtypes.bfloat16) for b in range(B)]
    # weight blocks in [d, n] (lhsT) layout, bf16
    wT = {}
    for j in range(2):
        sl = slice(j * HALF, (j + 1) * HALF)
        wT[j] = np.concatenate(
            [W[sl].T.astype(ml_dtypes.bfloat16) for W in (WS, WB4, WC)], axis=1)

    def bcol(bias_vec, j):
        return bias_vec[j * HALF:(j + 1) * HALF].reshape(NN, P).T  # [128, NN]

    in_maps = []
    for c in range(8):
        b, j = c // 2, c % 2
        # contraction-chunk permutation: own channel-half chunks first
        perm = list(range(4 * j, 4 * j + 4)) + list(range(4 * (1 - j), 4 * (1 - j) + 4))
        row_perm = np.concatenate([np.arange(k * P, (k + 1) * P) for k in perm])
        bias_m = np.concatenate(
            [bcol(v, j) for v in (bS, bB4, bC)], axis=1).astype(np.float32)
        in_maps.append({"xin": np.ascontiguousarray(xT_bf[b][row_perm]),
                        "win": np.ascontiguousarray(wT[j][row_perm]),
                        "bias": np.ascontiguousarray(bias_m)})
    return in_maps


def kernel(**inputs) -> np.ndarray:
    from concourse.bass_utils import run_bass_kernel_spmd

    _install_ldw_dedup()
    # accept np or jax arrays
    inputs = {k: np.asarray(v) for k, v in inputs.items()}

    if "nc" not in _STATE:
        _STATE["nc"] = _build_nc()
    nc = _STATE["nc"]

    in_maps = _prep_in_maps(inputs)
    res = run_bass_kernel_spmd(nc, in_maps, core_ids=list(range(8)),
                               trace=bool(_STATE.get("trace")))
    _STATE["last_results"] = res

    out = np.empty((B, L, N), np.float32)
    for c in range(8):
        b, j = c // 2, c % 2
        out[b, :, j * HALF:(j + 1) * HALF] = res.results[c]["y"].T
    return out
